# revision 17
# baseline (speedup 1.0000x reference)
"""EnsembleGATDGFLayer Trainium2 kernel.

Data-parallel over batch: 64 graphs -> 8 NeuronCores, 8 graphs each.
All layout prep (transposes, weight folding) happens on host; the device
kernel is pure matmul + elementwise with zero on-chip transposes.

Math (per graph, N=512 nodes, D=256 feat, P=64 op-emb):
  dense = gate_d * (adj @ (X@W)) + X@W + b      (DenseGraphFlow)
  scores = X @ M @ X.T,  M = Wq.T diag(a_w) Wk / 16
  attn = softmax(leaky_relu(scores) * adj)
  gat = LN(gate_g * attn @ (X@Wv.T)) * g + b2   (GraphAttention)
  out = 0.5*(dense + gat)

Device layout choices:
  - XT [D, N] resident (host-transposed), feeds every matmul as lhsT slices
    or rhs.
  - scores computed TRANSPOSED [l, e]: softmax runs over the partition dim
    implicitly — exp stays unnormalized, the softmax denominator S[e] comes
    from a ones-column appended to the Whv rhs of the attn@Whv matmul, and
    1/S is applied as a per-partition scalar afterwards.  This makes adj
    needed only in transposed form (host-provided) and attn never
    transposed at all.
"""

import numpy as np

B, N, DIN, DOUT, DOP = 64, 512, 256, 256, 64
NCORES = 8
G = B // NCORES
LN_EPS = 1e-5
NEG = 0.2

_BUILT = {}


def build_bass(g=G, mm_dt_name="float32r", apply_lng=False):
    """Build the per-core Bass module processing `g` graphs."""
    key = (g, mm_dt_name, apply_lng)
    if key in _BUILT:
        return _BUILT[key]

    import concourse.bass as bass
    import concourse.tile as tile
    from concourse import bacc, mybir

    f32 = mybir.dt.float32
    fmm = getattr(mybir.dt, mm_dt_name)
    AF = mybir.ActivationFunctionType
    OP = mybir.AluOpType

    nc = bacc.Bacc(None, target_bir_lowering=False, debug=False)

    # -------- DRAM I/O --------
    xt_d = nc.dram_tensor("xt", [g, 2, 128, N], fmm, kind="ExternalInput")
    adjt_d = nc.dram_tensor("adjt", [g, 4, 128, N], fmm, kind="ExternalInput")
    eta_d = nc.dram_tensor("eta", [g, 65, N], fmm, kind="ExternalInput")
    wsup_d = nc.dram_tensor("wsup", [2, 128, DOUT], fmm, kind="ExternalInput")
    wv_d = nc.dram_tensor("wv", [2, 128, DOUT], fmm, kind="ExternalInput")
    mq_d = nc.dram_tensor("mq", [2, 128, DIN], fmm, kind="ExternalInput")
    dgo_d = nc.dram_tensor("dgo", [65, DOUT], fmm, kind="ExternalInput")
    ggo_d = nc.dram_tensor("ggo", [65, DOUT], fmm, kind="ExternalInput")
    ch_d = nc.dram_tensor("chalf", [1, DOUT], f32, kind="ExternalInput")
    lng_d = nc.dram_tensor("lngh", [1, DOUT], f32, kind="ExternalInput")
    out_d = nc.dram_tensor("out", [g, 4, 128, DOUT], f32, kind="ExternalOutput")

    def mm(out_ap, lhsT, rhs, **kw):
        nc.tensor.matmul(out_ap, lhsT, rhs, **kw)

    with tile.TileContext(nc) as tc:
        with (
            tc.tile_pool(name="const", bufs=1) as cpool,
            tc.tile_pool(name="work", bufs=2) as wpool,
            tc.tile_pool(name="ps", bufs=8, space="PSUM") as ps,
        ):
            # -------- replicated params (loaded once) --------
            wsup_t = cpool.tile([128, 2, DOUT], fmm)
            nc.sync.dma_start(out=wsup_t[:], in_=wsup_d[:].rearrange("c p m -> p c m"))
            wv_t = cpool.tile([128, 2, DOUT], fmm)
            nc.sync.dma_start(out=wv_t[:], in_=wv_d[:].rearrange("c p m -> p c m"))
            mq_t = cpool.tile([128, 2, DIN], fmm)
            nc.sync.dma_start(out=mq_t[:], in_=mq_d[:].rearrange("c p m -> p c m"))
            dgo_t = cpool.tile([65, DOUT], fmm)
            nc.sync.dma_start(out=dgo_t[:], in_=dgo_d[:])
            ggo_t = cpool.tile([65, DOUT], fmm)
            nc.sync.dma_start(out=ggo_t[:], in_=ggo_d[:])
            cb_t = cpool.tile([128, DOUT], f32)
            nc.sync.dma_start(out=cb_t[:], in_=ch_d[:].to_broadcast([128, DOUT]))
            eps_t = cpool.tile([128, 1], f32)
            nc.vector.memset(eps_t[:], LN_EPS if apply_lng else 4.0 * LN_EPS)
            if apply_lng:
                lng_t = cpool.tile([128, DOUT], f32)
                nc.sync.dma_start(out=lng_t[:], in_=lng_d[:].to_broadcast([128, DOUT]))

            for gi in range(g):
                # -------- loads --------
                xt = wpool.tile([128, 2, N], fmm)
                nc.sync.dma_start(out=xt[:], in_=xt_d[gi].rearrange("c p n -> p c n"))
                adjt = wpool.tile([128, 4, N], fmm)
                nc.sync.dma_start(
                    out=adjt[:], in_=adjt_d[gi].rearrange("c p n -> p c n")
                )
                eta = wpool.tile([65, N], fmm)
                nc.sync.dma_start(out=eta[:], in_=eta_d[gi])

                # -------- support (X @ 0.5W), natural [l, m] --------
                sup_pl = wpool.tile([128, 4, DOUT], fmm)
                sup_c = wpool.tile([128, 4, DOUT], f32)
                for lc in range(4):
                    p = ps.tile([128, DOUT], f32, tag="ps")
                    for kc in range(2):
                        mm(p[:], xt[:, kc, lc * 128:(lc + 1) * 128], wsup_t[:, kc, :],
                           start=(kc == 0), stop=(kc == 1))
                    nc.scalar.copy(out=sup_pl[:, lc, :], in_=p[:])
                    nc.vector.tensor_add(out=sup_c[:, lc, :], in0=p[:], in1=cb_t[:])

                # -------- Whv (X @ Wv.T), natural [l, m], +ones col --------
                wvag = wpool.tile([128, 4, DOUT + 2], fmm)
                nc.gpsimd.memset(wvag[:, :, DOUT:DOUT + 2].bitcast(f32), 1.0)
                for lc in range(4):
                    p = ps.tile([128, DOUT], f32, tag="ps")
                    for kc in range(2):
                        mm(p[:], xt[:, kc, lc * 128:(lc + 1) * 128], wv_t[:, kc, :],
                           start=(kc == 0), stop=(kc == 1))
                    nc.scalar.copy(out=wvag[:, lc, :DOUT], in_=p[:])

                # -------- YT = M.T @ XT  [d', e] --------
                yt = wpool.tile([128, 2, N], fmm)
                for mc in range(2):
                    p = ps.tile([128, N], f32, tag="ps")
                    for kc in range(2):
                        mm(p[:], mq_t[:, kc, mc * 128:(mc + 1) * 128], xt[:, kc, :],
                           start=(kc == 0), stop=(kc == 1))
                    nc.scalar.copy(out=yt[:, mc, :], in_=p[:])

                # -------- scoresT [l, e] = X @ YT ; mask; leaky; exp --------
                # adj >= 0, so leaky(s)*adj == leaky(s*adj): mask first
                # (one PSUM operand), then leaky_relu(x) = max(0.2x, x).
                al = wpool.tile([128, 4, N], f32)
                for lc in range(4):
                    p = ps.tile([128, N], f32, tag="ps")
                    for kc in range(2):
                        mm(p[:], xt[:, kc, lc * 128:(lc + 1) * 128], yt[:, kc, :],
                           start=(kc == 0), stop=(kc == 1))
                    nc.vector.tensor_mul(out=al[:, lc, :], in0=p[:],
                                         in1=adjt[:, lc, :].bitcast(f32))
                lk = wpool.tile([128, 4, N], f32)
                ex = wpool.tile([128, 4, N], fmm)
                for h2 in range(2):
                    s = slice(h2 * 2, h2 * 2 + 2)
                    nc.vector.scalar_tensor_tensor(
                        out=lk[:, s, :], in0=al[:, s, :], scalar=NEG,
                        in1=al[:, s, :], op0=OP.mult, op1=OP.max)
                    nc.scalar.activation(out=ex[:, s, :], in_=lk[:, s, :], func=AF.Exp)

                # -------- AS = adjT.T @ support, natural [e, m]; dense --------
                gd = wpool.tile([128, 4, DOUT], f32)
                dn = wpool.tile([128, 4, DOUT], f32)
                for ec in range(4):
                    pg = ps.tile([128, DOUT], f32, tag="ps")
                    mm(pg[:], eta[:, ec * 128:(ec + 1) * 128], dgo_t[:],
                       start=True, stop=True)
                    nc.scalar.activation(out=gd[:, ec, :], in_=pg[:], func=AF.Sigmoid)
                    p = ps.tile([128, DOUT], f32, tag="ps")
                    for lc in range(4):
                        mm(p[:], adjt[:, lc, ec * 128:(ec + 1) * 128], sup_pl[:, lc, :],
                           start=(lc == 0), stop=(lc == 3))
                    nc.vector.tensor_mul(out=dn[:, ec, :], in0=p[:], in1=gd[:, ec, :])
                nc.gpsimd.tensor_add(out=dn[:], in0=dn[:], in1=sup_c[:])

                # -------- gate_g; h = gate_g * (exp @ WvAug) / S --------
                gg = wpool.tile([128, 4, DOUT], f32)
                h = wpool.tile([128, 4, DOUT], f32)
                rs = wpool.tile([128, 4, 1], f32)
                for ec in range(4):
                    pg = ps.tile([128, DOUT], f32, tag="ps")
                    mm(pg[:], eta[:, ec * 128:(ec + 1) * 128], ggo_t[:],
                       start=True, stop=True)
                    nc.scalar.activation(out=gg[:, ec, :], in_=pg[:], func=AF.Sigmoid)
                    p = ps.tile([128, DOUT + 2], f32, tag="ps")
                    for lc in range(4):
                        mm(p[:], ex[:, lc, ec * 128:(ec + 1) * 128], wvag[:, lc, :],
                           start=(lc == 0), stop=(lc == 3))
                    nc.vector.reciprocal(out=rs[:, ec, :], in_=p[:, DOUT:DOUT + 1])
                    nc.vector.scalar_tensor_tensor(
                        out=h[:, ec, :], in0=p[:, :DOUT], scalar=rs[:, ec, :],
                        in1=gg[:, ec, :], op0=OP.mult, op1=OP.mult)

                # -------- LayerNorm over m; final --------
                stats = wpool.tile([128, 4, 6], f32)
                mv = wpool.tile([128, 4, 2], f32)
                for ec in range(4):
                    nc.vector.bn_stats(out=stats[:, ec, :], in_=h[:, ec, :])
                    nc.vector.bn_aggr(out=mv[:, ec, :], in_=stats[:, ec, :])
                sd = wpool.tile([128, 4, 1], f32)
                rstd = wpool.tile([128, 4, 1], f32)
                # apply_lng: full rstd, 0.5*ln_g carries the halving.
                # else: rstd_half = 1/sqrt(4*var + 4*eps) = 0.5/sqrt(var+eps)
                nc.scalar.activation(out=sd[:], in_=mv[:, :, 1:2], func=AF.Sqrt,
                                     bias=eps_t[:],
                                     scale=1.0 if apply_lng else 4.0)
                nc.vector.reciprocal(out=rstd[:], in_=sd[:])
                t = wpool.tile([128, 4, DOUT], f32)
                for ec in range(4):
                    nc.gpsimd.tensor_scalar(
                        out=t[:, ec, :], in0=h[:, ec, :],
                        scalar1=mv[:, ec, 0:1], scalar2=rstd[:, ec, :],
                        op0=OP.subtract, op1=OP.mult)
                fin = wpool.tile([128, 4, DOUT], f32)
                if apply_lng:
                    for ec in range(4):
                        nc.gpsimd.tensor_mul(out=t[:, ec, :], in0=t[:, ec, :],
                                             in1=lng_t[:])
                nc.gpsimd.tensor_add(out=fin[:], in0=t[:], in1=dn[:])
                nc.sync.dma_start(out=out_d[gi].rearrange("c p m -> p c m"),
                                  in_=fin[:])

    nc.compile()
    _BUILT[key] = nc
    return nc


def tf32_round(a):
    """Round-to-nearest-even fp32 -> tf32 (10-bit mantissa) == fp32r."""
    u = np.ascontiguousarray(a, np.float32).view(np.uint32)
    u = (u + np.uint32(0x0FFF) + ((u >> np.uint32(13)) & np.uint32(1))) \
        & np.uint32(0xFFFFE000)
    return u.view(np.float32)


def prep_host(inputs, adj, op_emb, dgf_W, dgf_b, dgf_opW, dgf_opb,
              Wk, Wv, Wq, a_w, gat_opW, gat_opb, ln_g, ln_b):
    """Fold params + lay out per-graph tensors for the device kernel."""
    f = np.float32
    x = np.asarray(inputs, f)
    adj = np.asarray(adj, f)
    ope = np.asarray(op_emb, f)
    nb = x.shape[0]

    xt = np.ascontiguousarray(x.transpose(0, 2, 1)).reshape(nb, 2, 128, N)
    adjt = np.ascontiguousarray(adj.transpose(0, 2, 1)).reshape(nb, 4, 128, N)
    et = np.ascontiguousarray(ope.transpose(0, 2, 1))  # [nb, 64, N]
    eta = np.concatenate([et, np.ones((nb, 1, N), f)], axis=1)  # [nb, 65, N]

    wsup = np.ascontiguousarray(0.5 * np.asarray(dgf_W, f)).reshape(2, 128, DOUT)
    wvt = np.ascontiguousarray(np.asarray(Wv, f).T).reshape(2, 128, DOUT)
    mq = np.ascontiguousarray(
        (np.asarray(Wq, f).T * np.asarray(a_w, f)[None, :]) @ np.asarray(Wk, f)
        / np.sqrt(np.float32(DOUT))).reshape(2, 128, DIN)
    dgo = np.ascontiguousarray(
        np.concatenate([np.asarray(dgf_opW, f).T,
                        np.asarray(dgf_opb, f)[None, :]], 0))
    ggo = np.ascontiguousarray(
        np.concatenate([np.asarray(gat_opW, f).T,
                        np.asarray(gat_opb, f)[None, :]], 0))
    ch = np.ascontiguousarray(
        (0.5 * (np.asarray(dgf_b, f) + np.asarray(ln_b, f))).reshape(1, DOUT))
    lng = np.ascontiguousarray((0.5 * np.asarray(ln_g, f)).reshape(1, DOUT))
    apply_lng = not (np.all(np.asarray(ln_g, f) == 1.0))
    hp = dict(xt=xt, adjt=adjt, eta=eta, wsup=wsup, wv=wvt, mq=mq,
              dgo=dgo, ggo=ggo, chalf=ch, lngh=lng)
    if MM_DT == "float32r":
        # matmul-feeding tensors must carry fp32r(=tf32)-rounded values
        for k in ("xt", "adjt", "eta", "wsup", "wv", "mq", "dgo", "ggo"):
            hp[k] = tf32_round(hp[k])
    return hp, apply_lng


MM_DT = "float32r"


def run(hp, apply_lng, mm_dt=None, trace=False, **kw):
    from concourse.bass_utils import run_bass_kernel_spmd

    nc = build_bass(G, mm_dt or MM_DT, apply_lng)
    in_maps = []
    for c in range(NCORES):
        sl = slice(c * G, (c + 1) * G)
        m = {k: (v[sl] if k in ("xt", "adjt", "eta") else v)
             for k, v in hp.items()}
        in_maps.append(m)
    res = run_bass_kernel_spmd(nc, in_maps, core_ids=list(range(NCORES)),
                               trace=trace, **kw)
    out = np.concatenate(
        [r["out"].reshape(G, N, DOUT) for r in res.results], axis=0)
    return np.ascontiguousarray(out), res


def kernel(**inputs) -> np.ndarray:
    hp, apply_lng = prep_host(**inputs)
    out, _ = run(hp, apply_lng)
    return out


# revision 19
# speedup vs baseline: 1.2263x; 1.2263x over previous
"""EnsembleGATDGFLayer Trainium2 kernel.

Data-parallel over batch: 64 graphs -> 8 NeuronCores, 8 graphs each.
All layout prep (transposes, weight folding) happens on host; the device
kernel is pure matmul + elementwise with zero on-chip transposes.

Math (per graph, N=512 nodes, D=256 feat, P=64 op-emb):
  dense = gate_d * (adj @ (X@W)) + X@W + b      (DenseGraphFlow)
  scores = X @ M @ X.T,  M = Wq.T diag(a_w) Wk / 16
  attn = softmax(leaky_relu(scores) * adj)
  gat = LN(gate_g * attn @ (X@Wv.T)) * g + b2   (GraphAttention)
  out = 0.5*(dense + gat)

Device layout / engine choices:
  - XT [D, N] resident (host-transposed); all matmuls fp32r (tf32, 4x fp32).
  - scores computed TRANSPOSED [l, e]; softmax denominator S[e] comes from a
    2.0-column appended to the Whv rhs of the attn@Whv matmul (giving 2S and
    hence 1/(2S) = 0.5/S in one DVE reciprocal); exp stays unnormalized.
  - sigmoid(x) == 0.5*tanh(x/2)+0.5: gates use ACT Tanh so every ACT func
    (Copy/Identity/Exp/Tanh) lives in ONE act table set -> no table reloads.
    The (t+1) and *0.5 factors fold into scalar_tensor_tensor consumers and
    pre-scaled weights (0.25*dgf_W).
  - rstd via Quake rsqrt + 2 Newton steps on DVE (no ACT Sqrt).
  - LN apply on ACT: Identity(scale=rstd, bias=-mu*rstd).
"""

import numpy as np

B, N, DIN, DOUT, DOP = 64, 512, 256, 256, 64
NCORES = 8
G = B // NCORES
LN_EPS = 1e-5
NEG = 0.2
QMAGIC = 0x5F3759DF

_BUILT = {}


def build_bass(g=G, mm_dt_name="float32r", apply_lng=False):
    """Build the per-core Bass module processing `g` graphs."""
    key = (g, mm_dt_name, apply_lng)
    if key in _BUILT:
        return _BUILT[key]

    import concourse.bass as bass
    import concourse.tile as tile
    from concourse import bacc, mybir

    f32 = mybir.dt.float32
    i32 = mybir.dt.int32
    fmm = getattr(mybir.dt, mm_dt_name)
    AF = mybir.ActivationFunctionType
    OP = mybir.AluOpType

    nc = bacc.Bacc(None, target_bir_lowering=False, debug=False)

    # -------- DRAM I/O --------
    xt_d = nc.dram_tensor("xt", [g, 2, 128, N], fmm, kind="ExternalInput")
    adjt_d = nc.dram_tensor("adjt", [g, 4, 128, N], fmm, kind="ExternalInput")
    eta_d = nc.dram_tensor("eta", [g, 65, N], fmm, kind="ExternalInput")
    wsup_d = nc.dram_tensor("wsup", [2, 128, DOUT], fmm, kind="ExternalInput")
    wv_d = nc.dram_tensor("wv", [2, 128, DOUT], fmm, kind="ExternalInput")
    mq_d = nc.dram_tensor("mq", [2, 128, DIN], fmm, kind="ExternalInput")
    dgo_d = nc.dram_tensor("dgo", [65, DOUT], fmm, kind="ExternalInput")
    ggo_d = nc.dram_tensor("ggo", [65, DOUT], fmm, kind="ExternalInput")
    ch_d = nc.dram_tensor("chalf", [1, DOUT], f32, kind="ExternalInput")
    lng_d = nc.dram_tensor("lngh", [1, DOUT], f32, kind="ExternalInput")
    out_d = nc.dram_tensor("out", [g, 4, 128, DOUT], f32, kind="ExternalOutput")

    mm = nc.tensor.matmul

    with tile.TileContext(nc) as tc:
        with (
            tc.tile_pool(name="const", bufs=1) as cpool,
            tc.tile_pool(name="work", bufs=2) as wpool,
            tc.tile_pool(name="ps2", bufs=2, space="PSUM") as ps2,
            tc.tile_pool(name="ps1", bufs=4, space="PSUM") as ps1,
        ):
            # -------- replicated params (loaded once) --------
            wsup_t = cpool.tile([128, 2, DOUT], fmm)
            nc.sync.dma_start(out=wsup_t[:], in_=wsup_d[:].rearrange("c p m -> p c m"))
            wv_t = cpool.tile([128, 2, DOUT], fmm)
            nc.sync.dma_start(out=wv_t[:], in_=wv_d[:].rearrange("c p m -> p c m"))
            mq_t = cpool.tile([128, 2, DIN], fmm)
            nc.sync.dma_start(out=mq_t[:], in_=mq_d[:].rearrange("c p m -> p c m"))
            dgo_t = cpool.tile([65, DOUT], fmm)
            nc.sync.dma_start(out=dgo_t[:], in_=dgo_d[:])
            ggo_t = cpool.tile([65, DOUT], fmm)
            nc.sync.dma_start(out=ggo_t[:], in_=ggo_d[:])
            cb_t = cpool.tile([128, DOUT], f32)
            nc.sync.dma_start(out=cb_t[:], in_=ch_d[:].to_broadcast([128, DOUT]))
            if apply_lng:
                lng_t = cpool.tile([128, DOUT], f32)
                nc.sync.dma_start(out=lng_t[:], in_=lng_d[:].to_broadcast([128, DOUT]))

            for gi in range(g):
                # -------- loads --------
                xt = wpool.tile([128, 2, N], fmm)
                nc.sync.dma_start(out=xt[:], in_=xt_d[gi].rearrange("c p n -> p c n"))
                adjt = wpool.tile([128, 4, N], fmm)
                nc.sync.dma_start(
                    out=adjt[:], in_=adjt_d[gi].rearrange("c p n -> p c n")
                )
                eta = wpool.tile([65, N], fmm)
                nc.sync.dma_start(out=eta[:], in_=eta_d[gi])

                # -------- support (X @ 0.25W), natural [l, m] --------
                sp_ps = ps2.tile([128, 4, DOUT], f32, tag="ps2")
                for lc in range(4):
                    for kc in range(2):
                        mm(sp_ps[:, lc, :], xt[:, kc, lc * 128:(lc + 1) * 128],
                           wsup_t[:, kc, :], start=(kc == 0), stop=(kc == 1))
                sup_pl = wpool.tile([128, 4, DOUT], fmm)
                nc.scalar.copy(out=sup_pl[:], in_=sp_ps[:])
                # 0.5*support + c  ==  2*sp + c
                sup_c = wpool.tile([128, 4, DOUT], f32)
                cb_ap = cb_t[:]
                cb_rep = bass.AP(tensor=cb_ap.tensor, offset=cb_ap.offset,
                                 ap=[cb_ap.ap[0], [0, 4], cb_ap.ap[1]])
                nc.vector.scalar_tensor_tensor(
                    out=sup_c[:], in0=sp_ps[:], scalar=2.0,
                    in1=cb_rep, op0=OP.mult, op1=OP.add)

                # -------- Whv (X @ Wv.T), natural [l, m], cols 256/257 = 2.0 --------
                wv_ps = ps2.tile([128, 4, DOUT], f32, tag="ps2")
                for lc in range(4):
                    for kc in range(2):
                        mm(wv_ps[:, lc, :], xt[:, kc, lc * 128:(lc + 1) * 128],
                           wv_t[:, kc, :], start=(kc == 0), stop=(kc == 1))
                wvag = wpool.tile([128, 4, DOUT + 2], fmm)
                nc.gpsimd.memset(wvag[:, :, DOUT:DOUT + 2].bitcast(f32), 2.0)
                nc.scalar.copy(out=wvag[:, :, :DOUT], in_=wv_ps[:])

                # -------- YT = M.T @ XT  [d', e] --------
                yt_ps = ps2.tile([128, 2, N], f32, tag="ps2")
                for mc in range(2):
                    for kc in range(2):
                        mm(yt_ps[:, mc, :], mq_t[:, kc, mc * 128:(mc + 1) * 128],
                           xt[:, kc, :], start=(kc == 0), stop=(kc == 1))
                yt = wpool.tile([128, 2, N], fmm)
                nc.scalar.copy(out=yt[:], in_=yt_ps[:])

                # -------- scoresT [l, e] = X @ YT ; mask; leaky; exp --------
                # adj >= 0, so leaky(s)*adj == leaky(s*adj): mask first
                # (one PSUM operand), then leaky_relu(x) = max(0.2x, x).
                al = wpool.tile([128, 4, N], f32)
                for lc in range(4):
                    p = ps1.tile([128, N], f32, tag="ps1")
                    for kc in range(2):
                        mm(p[:], xt[:, kc, lc * 128:(lc + 1) * 128], yt[:, kc, :],
                           start=(kc == 0), stop=(kc == 1))
                    nc.vector.tensor_mul(out=al[:, lc, :], in0=p[:],
                                         in1=adjt[:, lc, :].bitcast(f32))
                lk = wpool.tile([128, 4, N], f32)
                ex = wpool.tile([128, 4, N], fmm)
                for h2 in range(2):
                    s = slice(h2 * 2, h2 * 2 + 2)
                    nc.vector.scalar_tensor_tensor(
                        out=lk[:, s, :], in0=al[:, s, :], scalar=NEG,
                        in1=al[:, s, :], op0=OP.mult, op1=OP.max)
                    nc.scalar.activation(out=ex[:, s, :], in_=lk[:, s, :], func=AF.Exp)

                # -------- gates: sigmoid(x) = 0.5*tanh(x/2) + 0.5 --------
                gd_ps = ps2.tile([128, 4, DOUT], f32, tag="ps2")
                for ec in range(4):
                    mm(gd_ps[:, ec, :], eta[:, ec * 128:(ec + 1) * 128], dgo_t[:],
                       start=True, stop=True)
                thd = wpool.tile([128, 4, DOUT], f32)
                nc.scalar.activation(out=thd[:], in_=gd_ps[:], func=AF.Tanh, scale=0.5)
                gg_ps = ps2.tile([128, 4, DOUT], f32, tag="ps2")
                for ec in range(4):
                    mm(gg_ps[:, ec, :], eta[:, ec * 128:(ec + 1) * 128], ggo_t[:],
                       start=True, stop=True)
                thg = wpool.tile([128, 4, DOUT], f32)
                nc.scalar.activation(out=thg[:], in_=gg_ps[:], func=AF.Tanh, scale=0.5)

                # -------- AS = adjT.T @ sup_pl, natural [e, m]; dense --------
                as_ps = ps2.tile([128, 4, DOUT], f32, tag="ps2")
                for ec in range(4):
                    for lc in range(4):
                        mm(as_ps[:, ec, :], adjt[:, lc, ec * 128:(ec + 1) * 128],
                           sup_pl[:, lc, :], start=(lc == 0), stop=(lc == 3))
                # dense = 0.5*gate_d*(adj@sup) + 0.5*sup + c = (thd+1)*AS + sup_c
                dn = wpool.tile([128, 4, DOUT], f32)
                nc.vector.scalar_tensor_tensor(
                    out=dn[:], in0=thd[:], scalar=1.0, in1=as_ps[:],
                    op0=OP.add, op1=OP.mult)
                nc.gpsimd.tensor_add(out=dn[:], in0=dn[:], in1=sup_c[:])

                # -------- h = gate_g * (exp @ Whv) / S --------
                t1 = wpool.tile([128, 4, DOUT], f32)
                rs = wpool.tile([128, 4, 1], f32)
                for ec in range(4):
                    p = ps1.tile([128, DOUT + 2], f32, tag="ps1")
                    for lc in range(4):
                        mm(p[:], ex[:, lc, ec * 128:(ec + 1) * 128], wvag[:, lc, :],
                           start=(lc == 0), stop=(lc == 3))
                    # col DOUT = 2*S  ->  rs = 1/(2S) = 0.5/S
                    nc.vector.reciprocal(out=rs[:, ec, :], in_=p[:, DOUT:DOUT + 1])
                    nc.scalar.activation(out=t1[:, ec, :], in_=p[:, :DOUT],
                                         func=AF.Copy, scale=rs[:, ec, :])
                h = wpool.tile([128, 4, DOUT], f32)
                nc.vector.scalar_tensor_tensor(
                    out=h[:], in0=thg[:], scalar=1.0, in1=t1[:],
                    op0=OP.add, op1=OP.mult)

                # -------- LayerNorm over m --------
                stats = wpool.tile([128, 4, 6], f32)
                mv = wpool.tile([128, 4, 2], f32)
                for ec in range(4):
                    nc.vector.bn_stats(out=stats[:, ec, :], in_=h[:, ec, :])
                    nc.vector.bn_aggr(out=mv[:, ec, :], in_=stats[:, ec, :])
                # rstd (or rstd/2) via Quake rsqrt + 2 Newton steps, all DVE.
                # w = 4*(var+eps) gives rsqrt(w) = 0.5/sqrt(var+eps).
                w = wpool.tile([128, 4, 1], f32)
                sc0 = 1.0 if apply_lng else 4.0
                nc.vector.tensor_scalar(
                    out=w[:], in0=mv[:, :, 1:2], scalar1=sc0,
                    scalar2=sc0 * LN_EPS, op0=OP.mult, op1=OP.add)
                yq = wpool.tile([128, 4, 1], f32)
                tq = wpool.tile([128, 4, 1], i32)
                nc.vector.tensor_scalar(
                    out=tq[:], in0=w[:].bitcast(i32), scalar1=1,
                    scalar2=None, op0=OP.arith_shift_right)
                nc.vector.tensor_scalar(
                    out=yq[:].bitcast(i32), in0=tq[:], scalar1=QMAGIC,
                    scalar2=-1, op0=OP.subtract, op1=OP.mult)
                aq = wpool.tile([128, 4, 1], f32)
                for _ in range(2):
                    nc.vector.tensor_mul(out=aq[:], in0=yq[:], in1=yq[:])
                    nc.vector.tensor_mul(out=aq[:], in0=aq[:], in1=w[:])
                    nc.vector.tensor_scalar(
                        out=aq[:], in0=aq[:], scalar1=-0.5, scalar2=1.5,
                        op0=OP.mult, op1=OP.add)
                    nc.vector.tensor_mul(out=yq[:], in0=yq[:], in1=aq[:])
                # nb = -mu * rstd
                nb = wpool.tile([128, 4, 1], f32)
                nc.vector.scalar_tensor_tensor(
                    out=nb[:], in0=mv[:, :, 0:1], scalar=-1.0, in1=yq[:],
                    op0=OP.mult, op1=OP.mult)
                t = wpool.tile([128, 4, DOUT], f32)
                for ec in range(4):
                    nc.scalar.activation(out=t[:, ec, :], in_=h[:, ec, :],
                                         func=AF.Identity, bias=nb[:, ec, :],
                                         scale=yq[:, ec, :])
                fin = wpool.tile([128, 4, DOUT], f32)
                if apply_lng:
                    for ec in range(4):
                        nc.gpsimd.tensor_mul(out=t[:, ec, :], in0=t[:, ec, :],
                                             in1=lng_t[:])
                nc.gpsimd.tensor_add(out=fin[:], in0=t[:], in1=dn[:])
                nc.sync.dma_start(out=out_d[gi].rearrange("c p m -> p c m"),
                                  in_=fin[:])

    nc.compile()
    _BUILT[key] = nc
    return nc


def tf32_round(a):
    """Round-to-nearest-even fp32 -> tf32 (10-bit mantissa) == fp32r."""
    u = np.ascontiguousarray(a, np.float32).view(np.uint32)
    u = (u + np.uint32(0x0FFF) + ((u >> np.uint32(13)) & np.uint32(1))) \
        & np.uint32(0xFFFFE000)
    return u.view(np.float32)


def prep_host(inputs, adj, op_emb, dgf_W, dgf_b, dgf_opW, dgf_opb,
              Wk, Wv, Wq, a_w, gat_opW, gat_opb, ln_g, ln_b):
    """Fold params + lay out per-graph tensors for the device kernel."""
    f = np.float32
    x = np.asarray(inputs, f)
    adj = np.asarray(adj, f)
    ope = np.asarray(op_emb, f)
    nb = x.shape[0]

    xt = np.ascontiguousarray(x.transpose(0, 2, 1)).reshape(nb, 2, 128, N)
    adjt = np.ascontiguousarray(adj.transpose(0, 2, 1)).reshape(nb, 4, 128, N)
    et = np.ascontiguousarray(ope.transpose(0, 2, 1))  # [nb, 64, N]
    eta = np.concatenate([et, np.ones((nb, 1, N), f)], axis=1)  # [nb, 65, N]

    wsup = np.ascontiguousarray(0.25 * np.asarray(dgf_W, f)).reshape(2, 128, DOUT)
    wvt = np.ascontiguousarray(np.asarray(Wv, f).T).reshape(2, 128, DOUT)
    mq = np.ascontiguousarray(
        (np.asarray(Wq, f).T * np.asarray(a_w, f)[None, :]) @ np.asarray(Wk, f)
        / np.sqrt(np.float32(DOUT))).reshape(2, 128, DIN)
    dgo = np.ascontiguousarray(
        np.concatenate([np.asarray(dgf_opW, f).T,
                        np.asarray(dgf_opb, f)[None, :]], 0))
    ggo = np.ascontiguousarray(
        np.concatenate([np.asarray(gat_opW, f).T,
                        np.asarray(gat_opb, f)[None, :]], 0))
    ch = np.ascontiguousarray(
        (0.5 * (np.asarray(dgf_b, f) + np.asarray(ln_b, f))).reshape(1, DOUT))
    lng = np.ascontiguousarray((0.5 * np.asarray(ln_g, f)).reshape(1, DOUT))
    apply_lng = not (np.all(np.asarray(ln_g, f) == 1.0))
    hp = dict(xt=xt, adjt=adjt, eta=eta, wsup=wsup, wv=wvt, mq=mq,
              dgo=dgo, ggo=ggo, chalf=ch, lngh=lng)
    if MM_DT == "float32r":
        # matmul-feeding tensors must carry fp32r(=tf32)-rounded values
        for k in ("xt", "adjt", "eta", "wsup", "wv", "mq", "dgo", "ggo"):
            hp[k] = tf32_round(hp[k])
    return hp, apply_lng


MM_DT = "float32r"


def run(hp, apply_lng, mm_dt=None, trace=False, **kw):
    from concourse.bass_utils import run_bass_kernel_spmd

    nc = build_bass(G, mm_dt or MM_DT, apply_lng)
    in_maps = []
    for c in range(NCORES):
        sl = slice(c * G, (c + 1) * G)
        m = {k: (v[sl] if k in ("xt", "adjt", "eta") else v)
             for k, v in hp.items()}
        in_maps.append(m)
    res = run_bass_kernel_spmd(nc, in_maps, core_ids=list(range(NCORES)),
                               trace=trace, **kw)
    out = np.concatenate(
        [r["out"].reshape(G, N, DOUT) for r in res.results], axis=0)
    return np.ascontiguousarray(out), res


def kernel(**inputs) -> np.ndarray:
    hp, apply_lng = prep_host(**inputs)
    out, _ = run(hp, apply_lng)
    return out


# revision 23
# speedup vs baseline: 1.3596x; 1.1087x over previous
"""EnsembleGATDGFLayer Trainium2 kernel.

Data-parallel over batch: 64 graphs -> 8 NeuronCores, 8 graphs each.
All layout prep (transposes, weight folding) happens on host; the device
kernel is pure matmul + elementwise with zero on-chip transposes.

Math (per graph, N=512 nodes, D=256 feat, P=64 op-emb):
  dense = gate_d * (adj @ (X@W)) + X@W + b      (DenseGraphFlow)
  scores = X @ M @ X.T,  M = Wq.T diag(a_w) Wk / 16
  attn = softmax(leaky_relu(scores) * adj)
  gat = LN(gate_g * attn @ (X@Wv.T)) * g + b2   (GraphAttention)
  out = 0.5*(dense + gat)

Key tricks:
  - All matmuls fp32r (tf32; 4x fp32 rate at free-dim >= 256); host
    pre-rounds matmul operands RNE to tf32.
  - scores computed TRANSPOSED [l, e] so adj is only needed transposed
    (host-provided) and attn (=exp, unnormalized) feeds matmuls directly.
  - softmax 1/S normalization is per-row positive -> cancels inside the
    downstream LayerNorm (scale invariance): never computed at all.
  - sigmoid(x) == 0.5*tanh(x/2)+0.5: gates use ACT Tanh so every ACT func
    lives in one act-table set -> no table reloads; the +1/x0.5 factors fold
    into scalar_tensor_tensor consumers and pre-scaled weights.
  - rhs packing: [0.25*dgf_W | Wv.T] and [dgf_opW.T+b | gat_opW.T+b] halve
    the support/Whv and gate matmul counts.
  - rstd via Quake rsqrt + 2 Newton steps on DVE (no ACT Sqrt).
  - per-graph emission is software-pipelined: front(g+1) before back(g) so
    the PE always has independent matmuls while exp/leaky cook.
"""

import os

import numpy as np

B, N, DIN, DOUT, DOP = 64, 512, 256, 256, 64
NCORES = 8
G = B // NCORES
LN_EPS = 1e-5
NEG = 0.2
QMAGIC = 0x5F3759DF
USE_PRELU = os.environ.get("USE_PRELU", "1") != "0"

_BUILT = {}


def build_bass(g=G, mm_dt_name="float32r", apply_lng=False, use_prelu=None):
    """Build the per-core Bass module processing `g` graphs."""
    if use_prelu is None:
        use_prelu = USE_PRELU
    key = (g, mm_dt_name, apply_lng, use_prelu)
    if key in _BUILT:
        return _BUILT[key]

    import concourse.bass as bass
    import concourse.tile as tile
    from concourse import bacc, mybir

    f32 = mybir.dt.float32
    i32 = mybir.dt.int32
    fmm = getattr(mybir.dt, mm_dt_name)
    AF = mybir.ActivationFunctionType
    OP = mybir.AluOpType

    nc = bacc.Bacc(None, target_bir_lowering=False, debug=False)

    # -------- DRAM I/O --------
    xt_d = nc.dram_tensor("xt", [g, 2, 128, N], fmm, kind="ExternalInput")
    adjt_d = nc.dram_tensor("adjt", [g, 4, 128, N], fmm, kind="ExternalInput")
    eta_d = nc.dram_tensor("eta", [g, 65, N], fmm, kind="ExternalInput")
    wc_d = nc.dram_tensor("wcomb", [2, 128, 512], fmm, kind="ExternalInput")
    mq_d = nc.dram_tensor("mq", [2, 128, DIN], fmm, kind="ExternalInput")
    go_d = nc.dram_tensor("gcomb", [65, 512], fmm, kind="ExternalInput")
    ch_d = nc.dram_tensor("chalf", [1, DOUT], f32, kind="ExternalInput")
    lng_d = nc.dram_tensor("lngh", [1, DOUT], f32, kind="ExternalInput")
    out_d = nc.dram_tensor("out", [g, 4, 128, DOUT], f32, kind="ExternalOutput")

    mm = nc.tensor.matmul

    with tile.TileContext(nc) as tc:
        with (
            tc.tile_pool(name="const", bufs=1) as cpool,
            tc.tile_pool(name="work", bufs=2) as wpool,
            tc.tile_pool(name="ps1", bufs=4, space="PSUM") as ps1,
            tc.tile_pool(name="ps2", bufs=2, space="PSUM") as ps2,
        ):
            # -------- replicated params --------
            mq_t = cpool.tile([128, 2, DIN], fmm)
            wc_t = cpool.tile([128, 2, 512], fmm)
            go_t = cpool.tile([65, 512], fmm)
            cb_t = cpool.tile([128, DOUT], f32)
            if apply_lng:
                lng_t = cpool.tile([128, DOUT], f32)

            def load_consts():
                nc.sync.dma_start(out=mq_t[:],
                                  in_=mq_d[:].rearrange("c p m -> p c m"))
                nc.sync.dma_start(out=wc_t[:],
                                  in_=wc_d[:].rearrange("c p m -> p c m"))
                nc.sync.dma_start(out=go_t[:], in_=go_d[:])
                nc.sync.dma_start(out=cb_t[:],
                                  in_=ch_d[:].to_broadcast([128, DOUT]))
                if apply_lng:
                    nc.sync.dma_start(out=lng_t[:],
                                      in_=lng_d[:].to_broadcast([128, DOUT]))

            def front(gi):
                """loads + projections + scores + exp + gates for graph gi."""
                st = {}
                xt = wpool.tile([128, 2, N], fmm, tag="xt")
                nc.sync.dma_start(out=xt[:],
                                  in_=xt_d[gi].rearrange("c p n -> p c n"))
                if gi == 0:
                    load_consts()
                adjt = wpool.tile([128, 4, N], fmm, tag="adjt")
                nc.sync.dma_start(out=adjt[:],
                                  in_=adjt_d[gi].rearrange("c p n -> p c n"))
                eta = wpool.tile([65, N], fmm, tag="eta")
                nc.sync.dma_start(out=eta[:], in_=eta_d[gi])

                # YT = M.T @ XT  [d', e]
                yt_ps = ps2.tile([128, 2, N], f32, tag="ps2")
                for mc in range(2):
                    for kc in range(2):
                        mm(yt_ps[:, mc, :], mq_t[:, kc, mc * 128:(mc + 1) * 128],
                           xt[:, kc, :], start=(kc == 0), stop=(kc == 1))
                yt = wpool.tile([128, 2, N], fmm, tag="yt")
                nc.scalar.copy(out=yt[:], in_=yt_ps[:])

                # [0.25*support | Whv | 2.0 2.0] = X-projections, natural [l, m]
                comb = wpool.tile([128, 4, 516], fmm, tag="comb")
                nc.gpsimd.memset(comb[:, :, 512:514].bitcast(f32), 2.0)
                sup_c = wpool.tile([128, 4, DOUT], f32, tag="sup_c")
                cb_ap = cb_t[:]
                for lc in range(4):
                    p = ps1.tile([128, 512], f32, tag="ps1")
                    for kc in range(2):
                        mm(p[:], xt[:, kc, lc * 128:(lc + 1) * 128], wc_t[:, kc, :],
                           start=(kc == 0), stop=(kc == 1))
                    nc.scalar.copy(out=comb[:, lc, :512], in_=p[:])
                    # 0.5*support + c == 2*(0.25*support) + c
                    nc.vector.scalar_tensor_tensor(
                        out=sup_c[:, lc, :], in0=p[:, :DOUT], scalar=2.0,
                        in1=cb_ap, op0=OP.mult, op1=OP.add)

                # scoresT [l, e] = X @ YT ; mask; leaky; exp
                al = wpool.tile([128, 4, N], f32, tag="al")
                for lc in range(4):
                    p = ps1.tile([128, N], f32, tag="ps1")
                    for kc in range(2):
                        mm(p[:], xt[:, kc, lc * 128:(lc + 1) * 128], yt[:, kc, :],
                           start=(kc == 0), stop=(kc == 1))
                    # adj >= 0 so leaky(s)*adj == leaky(s*adj): mask first
                    nc.vector.tensor_mul(out=al[:, lc, :], in0=p[:],
                                         in1=adjt[:, lc, :].bitcast(f32))
                ex = wpool.tile([128, 4, N], fmm, tag="ex")
                lk = wpool.tile([128, 4, N], f32, tag="lk")
                for h2 in range(2):
                    s = slice(h2 * 2, h2 * 2 + 2)
                    if use_prelu:
                        nc.scalar.activation(out=lk[:, s, :], in_=al[:, s, :],
                                             func=AF.Prelu, alpha=NEG)
                    else:
                        nc.vector.scalar_tensor_tensor(
                            out=lk[:, s, :], in0=al[:, s, :], scalar=NEG,
                            in1=al[:, s, :], op0=OP.mult, op1=OP.max)
                    nc.scalar.activation(out=ex[:, s, :], in_=lk[:, s, :],
                                         func=AF.Exp)

                # gates: [gate_d | gate_g] = sigmoid = 0.5*tanh(x/2)+0.5
                th = wpool.tile([128, 4, 512], f32, tag="th")
                for ec in range(4):
                    p = ps1.tile([128, 512], f32, tag="ps1")
                    mm(p[:], eta[:, ec * 128:(ec + 1) * 128], go_t[:],
                       start=True, stop=True)
                    nc.scalar.activation(out=th[:, ec, :], in_=p[:],
                                         func=AF.Tanh, scale=0.5)
                st.update(xt=xt, adjt=adjt, comb=comb, sup_c=sup_c, ex=ex, th=th)
                return st

            def back(gi, st):
                """AS + dense + attn@Whv + LN + out for graph gi."""
                adjt, comb, sup_c = st["adjt"], st["comb"], st["sup_c"]
                ex, th = st["ex"], st["th"]

                # AS = adjT.T @ (0.25*support), natural [e, m]
                as_ps = ps2.tile([128, 4, DOUT], f32, tag="ps2")
                for ec in range(4):
                    for lc in range(4):
                        mm(as_ps[:, ec, :], adjt[:, lc, ec * 128:(ec + 1) * 128],
                           comb[:, lc, :DOUT], start=(lc == 0), stop=(lc == 3))
                # dense = (tanh_d+1)*AS + (0.5*support + c)
                dn = wpool.tile([128, 4, DOUT], f32, tag="dn")
                nc.vector.scalar_tensor_tensor(
                    out=dn[:], in0=th[:, :, :DOUT], scalar=1.0, in1=as_ps[:],
                    op0=OP.add, op1=OP.mult)
                nc.gpsimd.tensor_add(out=dn[:], in0=dn[:], in1=sup_c[:])

                # v = (tanh_g+1) * (exp @ Whv)  (= 2S * gate_g*attn@Whv; the
                # positive per-row 2S factor cancels in the LayerNorm below,
                # except through eps -- corrected via the 2S column.)
                h = wpool.tile([128, 4, DOUT], f32, tag="h")
                scol = wpool.tile([128, 4, 1], f32, tag="scol")
                for ec in range(4):
                    p = ps1.tile([128, 258], f32, tag="ps1")
                    for lc in range(4):
                        mm(p[:], ex[:, lc, ec * 128:(ec + 1) * 128],
                           comb[:, lc, DOUT:DOUT + 258],
                           start=(lc == 0), stop=(lc == 3))
                    nc.vector.tensor_copy(out=scol[:, ec, :],
                                          in_=p[:, 256:257])
                    nc.vector.scalar_tensor_tensor(
                        out=h[:, ec, :], in0=th[:, ec, DOUT:], scalar=1.0,
                        in1=p[:, :DOUT], op0=OP.add, op1=OP.mult)

                # LayerNorm over m
                stats = wpool.tile([128, 4, 6], f32, tag="stats")
                mv = wpool.tile([128, 4, 2], f32, tag="mv")
                for ec in range(4):
                    nc.vector.bn_stats(out=stats[:, ec, :], in_=h[:, ec, :])
                    nc.vector.bn_aggr(out=mv[:, ec, :], in_=stats[:, ec, :])
                # rstd (or rstd/2) via Quake rsqrt + 2 Newton steps (DVE only)
                # w = sc0*(var_v + eps*(2S)^2); rsqrt(w) absorbs the 2S scale
                w = wpool.tile([128, 4, 1], f32, tag="w")
                s2 = wpool.tile([128, 4, 1], f32, tag="s2")
                sc0 = 1.0 if apply_lng else 4.0
                nc.vector.tensor_mul(out=s2[:], in0=scol[:], in1=scol[:])
                nc.vector.tensor_scalar(
                    out=w[:], in0=mv[:, :, 1:2], scalar1=sc0,
                    scalar2=None, op0=OP.mult)
                nc.vector.scalar_tensor_tensor(
                    out=w[:], in0=s2[:], scalar=sc0 * LN_EPS, in1=w[:],
                    op0=OP.mult, op1=OP.add)
                yq = wpool.tile([128, 4, 1], f32, tag="yq")
                tq = wpool.tile([128, 4, 1], i32, tag="tq")
                nc.vector.tensor_scalar(
                    out=tq[:], in0=w[:].bitcast(i32), scalar1=1,
                    scalar2=None, op0=OP.arith_shift_right)
                nc.vector.tensor_scalar(
                    out=yq[:].bitcast(i32), in0=tq[:], scalar1=QMAGIC,
                    scalar2=-1, op0=OP.subtract, op1=OP.mult)
                aq = wpool.tile([128, 4, 1], f32, tag="aq")
                for _ in range(2):
                    nc.vector.tensor_mul(out=aq[:], in0=yq[:], in1=yq[:])
                    nc.vector.scalar_tensor_tensor(
                        out=aq[:], in0=aq[:], scalar=-0.5, in1=w[:],
                        op0=OP.mult, op1=OP.mult)
                    nc.vector.scalar_tensor_tensor(
                        out=yq[:], in0=aq[:], scalar=1.5, in1=yq[:],
                        op0=OP.add, op1=OP.mult)
                # nb = -mu * rstd
                nb = wpool.tile([128, 4, 1], f32, tag="nb")
                nc.vector.scalar_tensor_tensor(
                    out=nb[:], in0=mv[:, :, 0:1], scalar=-1.0, in1=yq[:],
                    op0=OP.mult, op1=OP.mult)
                t = wpool.tile([128, 4, DOUT], f32, tag="t")
                for ec in range(4):
                    nc.scalar.activation(out=t[:, ec, :], in_=h[:, ec, :],
                                         func=AF.Identity, bias=nb[:, ec, :],
                                         scale=yq[:, ec, :])
                fin = wpool.tile([128, 4, DOUT], f32, tag="fin")
                if apply_lng:
                    for ec in range(4):
                        nc.gpsimd.tensor_mul(out=t[:, ec, :], in0=t[:, ec, :],
                                             in1=lng_t[:])
                nc.gpsimd.tensor_add(out=fin[:], in0=t[:], in1=dn[:])
                nc.sync.dma_start(out=out_d[gi].rearrange("c p m -> p c m"),
                                  in_=fin[:])

            # software pipeline: front(g+1) is emitted before back(g)
            prev = front(0)
            for gi in range(1, g):
                cur = front(gi)
                back(gi - 1, prev)
                prev = cur
            back(g - 1, prev)

    nc.compile()
    _BUILT[key] = nc
    return nc


def tf32_round(a):
    """Round-to-nearest-even fp32 -> tf32 (10-bit mantissa) == fp32r."""
    u = np.ascontiguousarray(a, np.float32).view(np.uint32)
    u = (u + np.uint32(0x0FFF) + ((u >> np.uint32(13)) & np.uint32(1))) \
        & np.uint32(0xFFFFE000)
    return u.view(np.float32)


def prep_host(inputs, adj, op_emb, dgf_W, dgf_b, dgf_opW, dgf_opb,
              Wk, Wv, Wq, a_w, gat_opW, gat_opb, ln_g, ln_b):
    """Fold params + lay out per-graph tensors for the device kernel."""
    f = np.float32
    x = np.asarray(inputs, f)
    adj = np.asarray(adj, f)
    ope = np.asarray(op_emb, f)
    nb = x.shape[0]

    xt = np.ascontiguousarray(x.transpose(0, 2, 1)).reshape(nb, 2, 128, N)
    adjt = np.ascontiguousarray(adj.transpose(0, 2, 1)).reshape(nb, 4, 128, N)
    et = np.ascontiguousarray(ope.transpose(0, 2, 1))  # [nb, 64, N]
    eta = np.concatenate([et, np.ones((nb, 1, N), f)], axis=1)  # [nb, 65, N]

    wcomb = np.ascontiguousarray(np.concatenate(
        [0.25 * np.asarray(dgf_W, f), np.asarray(Wv, f).T],
        axis=1)).reshape(2, 128, 512)
    mq = np.ascontiguousarray(
        (np.asarray(Wq, f).T * np.asarray(a_w, f)[None, :]) @ np.asarray(Wk, f)
        / np.sqrt(np.float32(DOUT))).reshape(2, 128, DIN)
    gcomb = np.ascontiguousarray(np.concatenate([
        np.concatenate([np.asarray(dgf_opW, f).T,
                        np.asarray(dgf_opb, f)[None, :]], 0),
        np.concatenate([np.asarray(gat_opW, f).T,
                        np.asarray(gat_opb, f)[None, :]], 0)], axis=1))
    ch = np.ascontiguousarray(
        (0.5 * (np.asarray(dgf_b, f) + np.asarray(ln_b, f))).reshape(1, DOUT))
    lng = np.ascontiguousarray((0.5 * np.asarray(ln_g, f)).reshape(1, DOUT))
    apply_lng = not (np.all(np.asarray(ln_g, f) == 1.0))
    hp = dict(xt=xt, adjt=adjt, eta=eta, wcomb=wcomb, mq=mq, gcomb=gcomb,
              chalf=ch, lngh=lng)
    if MM_DT == "float32r":
        # matmul-feeding tensors must carry fp32r(=tf32)-rounded values
        for k in ("xt", "adjt", "eta", "wcomb", "mq", "gcomb"):
            hp[k] = tf32_round(hp[k])
    return hp, apply_lng


MM_DT = "float32r"


def run(hp, apply_lng, mm_dt=None, trace=False, **kw):
    from concourse.bass_utils import run_bass_kernel_spmd

    nc = build_bass(G, mm_dt or MM_DT, apply_lng)
    in_maps = []
    for c in range(NCORES):
        sl = slice(c * G, (c + 1) * G)
        m = {k: (v[sl] if k in ("xt", "adjt", "eta") else v)
             for k, v in hp.items()}
        in_maps.append(m)
    res = run_bass_kernel_spmd(nc, in_maps, core_ids=list(range(NCORES)),
                               trace=trace, **kw)
    out = np.concatenate(
        [r["out"].reshape(G, N, DOUT) for r in res.results], axis=0)
    return np.ascontiguousarray(out), res


def kernel(**inputs) -> np.ndarray:
    hp, apply_lng = prep_host(**inputs)
    out, _ = run(hp, apply_lng)
    return out


# revision 26
# speedup vs baseline: 1.3830x; 1.0172x over previous
"""EnsembleGATDGFLayer Trainium2 kernel.

Data-parallel over batch: 64 graphs -> 8 NeuronCores, 8 graphs each.
All layout prep (transposes, weight folding) happens on host; the device
kernel is pure matmul + elementwise with zero on-chip transposes.

Math (per graph, N=512 nodes, D=256 feat, P=64 op-emb):
  dense = gate_d * (adj @ (X@W)) + X@W + b      (DenseGraphFlow)
  scores = X @ M @ X.T,  M = Wq.T diag(a_w) Wk / 16
  attn = softmax(leaky_relu(scores) * adj)
  gat = LN(gate_g * attn @ (X@Wv.T)) * g + b2   (GraphAttention)
  out = 0.5*(dense + gat)

Key tricks:
  - All matmuls fp32r (tf32; 4x fp32 rate at free-dim >= 256); host
    pre-rounds matmul operands RNE to tf32.
  - scores computed TRANSPOSED [l, e] so adj is only needed transposed
    (host-provided) and attn (=exp, unnormalized) feeds matmuls directly.
  - softmax 1/S normalization is per-row positive -> cancels inside the
    downstream LayerNorm (scale invariance): never computed at all.
  - sigmoid(x) == 0.5*tanh(x/2)+0.5: gates use ACT Tanh so every ACT func
    lives in one act-table set -> no table reloads; the +1/x0.5 factors fold
    into scalar_tensor_tensor consumers and pre-scaled weights.
  - rhs packing: [0.25*dgf_W | Wv.T] and [dgf_opW.T+b | gat_opW.T+b] halve
    the support/Whv and gate matmul counts.
  - rstd via Quake rsqrt + 2 Newton steps on DVE (no ACT Sqrt).
  - per-graph emission is software-pipelined: front(g+1) before back(g) so
    the PE always has independent matmuls while exp/leaky cook.
"""

import os

import numpy as np

B, N, DIN, DOUT, DOP = 64, 512, 256, 256, 64
NCORES = 8
G = B // NCORES
LN_EPS = 1e-5
NEG = 0.2
QMAGIC = 0x5F3759DF
USE_PRELU = os.environ.get("USE_PRELU", "1") != "0"

_BUILT = {}


def build_bass(g=G, mm_dt_name="float32r", apply_lng=False, use_prelu=None):
    """Build the per-core Bass module processing `g` graphs."""
    if use_prelu is None:
        use_prelu = USE_PRELU
    key = (g, mm_dt_name, apply_lng, use_prelu)
    if key in _BUILT:
        return _BUILT[key]

    import concourse.bass as bass
    import concourse.tile as tile
    from concourse import bacc, mybir

    f32 = mybir.dt.float32
    i32 = mybir.dt.int32
    fmm = getattr(mybir.dt, mm_dt_name)
    AF = mybir.ActivationFunctionType
    OP = mybir.AluOpType

    nc = bacc.Bacc(None, target_bir_lowering=False, debug=False)

    # -------- DRAM I/O --------
    xt_d = nc.dram_tensor("xt", [g, 2, 128, N], fmm, kind="ExternalInput")
    adjt_d = nc.dram_tensor("adjt", [g, 4, 128, N], fmm, kind="ExternalInput")
    eta_d = nc.dram_tensor("eta", [g, 65, N], fmm, kind="ExternalInput")
    wc_d = nc.dram_tensor("wcomb", [2, 128, 512], fmm, kind="ExternalInput")
    mq_d = nc.dram_tensor("mq", [2, 128, DIN], fmm, kind="ExternalInput")
    go_d = nc.dram_tensor("gcomb", [65, 512], fmm, kind="ExternalInput")
    ch_d = nc.dram_tensor("chalf", [1, DOUT], f32, kind="ExternalInput")
    lng_d = nc.dram_tensor("lngh", [1, DOUT], f32, kind="ExternalInput")
    out_d = nc.dram_tensor("out", [g, 4, 128, DOUT], f32, kind="ExternalOutput")

    mm = nc.tensor.matmul

    with tile.TileContext(nc) as tc:
        with (
            tc.tile_pool(name="const", bufs=1) as cpool,
            tc.tile_pool(name="work", bufs=2) as wpool,
            tc.tile_pool(name="ps1", bufs=6, space="PSUM") as ps1,
            tc.tile_pool(name="ps2", bufs=1, space="PSUM") as ps2,
        ):
            # -------- replicated params --------
            mq_t = cpool.tile([128, 2, DIN], fmm)
            wc_t = cpool.tile([128, 2, 512], fmm)
            go_t = cpool.tile([65, 512], fmm)
            cb_t = cpool.tile([128, DOUT], f32)
            if apply_lng:
                lng_t = cpool.tile([128, DOUT], f32)

            def load_consts():
                nc.sync.dma_start(out=mq_t[:],
                                  in_=mq_d[:].rearrange("c p m -> p c m"))
                nc.sync.dma_start(out=wc_t[:],
                                  in_=wc_d[:].rearrange("c p m -> p c m"))
                nc.sync.dma_start(out=go_t[:], in_=go_d[:])
                nc.sync.dma_start(out=cb_t[:],
                                  in_=ch_d[:].to_broadcast([128, DOUT]))
                if apply_lng:
                    nc.sync.dma_start(out=lng_t[:],
                                      in_=lng_d[:].to_broadcast([128, DOUT]))

            def loads(gi):
                xt = wpool.tile([128, 2, N], fmm, tag="xt", bufs=3)
                nc.sync.dma_start(out=xt[:],
                                  in_=xt_d[gi].rearrange("c p n -> p c n"))
                if gi == 0:
                    load_consts()
                adjt = wpool.tile([128, 4, N], fmm, tag="adjt", bufs=3)
                nc.sync.dma_start(out=adjt[:],
                                  in_=adjt_d[gi].rearrange("c p n -> p c n"))
                eta = wpool.tile([65, N], fmm, tag="eta", bufs=3)
                nc.sync.dma_start(out=eta[:], in_=eta_d[gi])
                return dict(xt=xt, adjt=adjt, eta=eta)

            def front(gi, st):
                """projections + scores + exp + gates for graph gi."""
                xt, adjt, eta = st["xt"], st["adjt"], st["eta"]

                # YT = M.T @ XT  [d', e]
                yt = wpool.tile([128, 2, N], fmm, tag="yt")
                for mc in range(2):
                    p = ps1.tile([128, N], f32, tag="ps1")
                    for kc in range(2):
                        mm(p[:], mq_t[:, kc, mc * 128:(mc + 1) * 128],
                           xt[:, kc, :], start=(kc == 0), stop=(kc == 1))
                    nc.scalar.copy(out=yt[:, mc, :], in_=p[:])

                # [0.25*support | Whv | 2.0 2.0] = X-projections, natural [l, m]
                comb = wpool.tile([128, 4, 516], fmm, tag="comb")
                nc.gpsimd.memset(comb[:, :, 512:514].bitcast(f32), 2.0)
                sup_c = wpool.tile([128, 4, DOUT], f32, tag="sup_c")
                cb_ap = cb_t[:]
                for lc in range(4):
                    p = ps1.tile([128, 512], f32, tag="ps1")
                    for kc in range(2):
                        mm(p[:], xt[:, kc, lc * 128:(lc + 1) * 128], wc_t[:, kc, :],
                           start=(kc == 0), stop=(kc == 1))
                    nc.scalar.copy(out=comb[:, lc, :512], in_=p[:])
                    # 0.5*support + c == 2*(0.25*support) + c
                    nc.vector.scalar_tensor_tensor(
                        out=sup_c[:, lc, :], in0=p[:, :DOUT], scalar=2.0,
                        in1=cb_ap, op0=OP.mult, op1=OP.add)

                # scoresT [l, e] = X @ YT ; mask; leaky; exp
                al = wpool.tile([128, 4, N], f32, tag="al")
                for lc in range(4):
                    p = ps1.tile([128, N], f32, tag="ps1")
                    for kc in range(2):
                        mm(p[:], xt[:, kc, lc * 128:(lc + 1) * 128], yt[:, kc, :],
                           start=(kc == 0), stop=(kc == 1))
                    # adj >= 0 so leaky(s)*adj == leaky(s*adj): mask first
                    nc.vector.tensor_mul(out=al[:, lc, :], in0=p[:],
                                         in1=adjt[:, lc, :].bitcast(f32))
                ex = wpool.tile([128, 4, N], fmm, tag="ex")
                lk = wpool.tile([128, 4, N], f32, tag="lk")
                for h2 in range(2):
                    s = slice(h2 * 2, h2 * 2 + 2)
                    if use_prelu:
                        nc.scalar.activation(out=lk[:, s, :], in_=al[:, s, :],
                                             func=AF.Prelu, alpha=NEG)
                    else:
                        nc.vector.scalar_tensor_tensor(
                            out=lk[:, s, :], in0=al[:, s, :], scalar=NEG,
                            in1=al[:, s, :], op0=OP.mult, op1=OP.max)
                    nc.scalar.activation(out=ex[:, s, :], in_=lk[:, s, :],
                                         func=AF.Exp)

                # gates: [gate_d | gate_g] = sigmoid = 0.5*tanh(x/2)+0.5
                th = wpool.tile([128, 4, 512], f32, tag="th")
                for ec in range(4):
                    p = ps1.tile([128, 512], f32, tag="ps1")
                    mm(p[:], eta[:, ec * 128:(ec + 1) * 128], go_t[:],
                       start=True, stop=True)
                    nc.scalar.activation(out=th[:, ec, :], in_=p[:],
                                         func=AF.Tanh, scale=0.5)
                st.update(comb=comb, sup_c=sup_c, ex=ex, th=th)
                return st

            def back(gi, st):
                """AS + dense + attn@Whv + LN + out for graph gi."""
                adjt, comb, sup_c = st["adjt"], st["comb"], st["sup_c"]
                ex, th = st["ex"], st["th"]

                # AS = adjT.T @ (0.25*support), natural [e, m]
                as_ps = ps2.tile([128, 4, DOUT], f32, tag="ps2")
                for ec in range(4):
                    for lc in range(4):
                        mm(as_ps[:, ec, :], adjt[:, lc, ec * 128:(ec + 1) * 128],
                           comb[:, lc, :DOUT], start=(lc == 0), stop=(lc == 3))
                # dense = (tanh_d+1)*AS + (0.5*support + c)
                dn = wpool.tile([128, 4, DOUT], f32, tag="dn")
                nc.vector.scalar_tensor_tensor(
                    out=dn[:], in0=th[:, :, :DOUT], scalar=1.0, in1=as_ps[:],
                    op0=OP.add, op1=OP.mult)
                nc.gpsimd.tensor_add(out=dn[:], in0=dn[:], in1=sup_c[:])

                # v = (tanh_g+1) * (exp @ Whv)  (= 2S * gate_g*attn@Whv; the
                # positive per-row 2S factor cancels in the LayerNorm below,
                # except through eps -- corrected via the 2S column.)
                h = wpool.tile([128, 4, DOUT], f32, tag="h")
                scol = wpool.tile([128, 4, 1], f32, tag="scol")
                for ec in range(4):
                    p = ps1.tile([128, 258], f32, tag="ps1")
                    for lc in range(4):
                        mm(p[:], ex[:, lc, ec * 128:(ec + 1) * 128],
                           comb[:, lc, DOUT:DOUT + 258],
                           start=(lc == 0), stop=(lc == 3))
                    nc.vector.tensor_copy(out=scol[:, ec, :],
                                          in_=p[:, 256:257])
                    nc.vector.scalar_tensor_tensor(
                        out=h[:, ec, :], in0=th[:, ec, DOUT:], scalar=1.0,
                        in1=p[:, :DOUT], op0=OP.add, op1=OP.mult)

                # LayerNorm over m
                stats = wpool.tile([128, 4, 6], f32, tag="stats")
                mv = wpool.tile([128, 4, 2], f32, tag="mv")
                for ec in range(4):
                    nc.vector.bn_stats(out=stats[:, ec, :], in_=h[:, ec, :])
                    nc.vector.bn_aggr(out=mv[:, ec, :], in_=stats[:, ec, :])
                # rstd (or rstd/2) via Quake rsqrt + 2 Newton steps (DVE only)
                # w = sc0*(var_v + eps*(2S)^2); rsqrt(w) absorbs the 2S scale
                w = wpool.tile([128, 4, 1], f32, tag="w")
                s2 = wpool.tile([128, 4, 1], f32, tag="s2")
                sc0 = 1.0 if apply_lng else 4.0
                nc.vector.tensor_mul(out=s2[:], in0=scol[:], in1=scol[:])
                nc.vector.tensor_scalar(
                    out=w[:], in0=mv[:, :, 1:2], scalar1=sc0,
                    scalar2=None, op0=OP.mult)
                nc.vector.scalar_tensor_tensor(
                    out=w[:], in0=s2[:], scalar=sc0 * LN_EPS, in1=w[:],
                    op0=OP.mult, op1=OP.add)
                yq = wpool.tile([128, 4, 1], f32, tag="yq")
                tq = wpool.tile([128, 4, 1], i32, tag="tq")
                nc.vector.tensor_scalar(
                    out=tq[:], in0=w[:].bitcast(i32), scalar1=1,
                    scalar2=None, op0=OP.arith_shift_right)
                nc.vector.tensor_scalar(
                    out=yq[:].bitcast(i32), in0=tq[:], scalar1=QMAGIC,
                    scalar2=-1, op0=OP.subtract, op1=OP.mult)
                aq = wpool.tile([128, 4, 1], f32, tag="aq")
                for _ in range(2):
                    nc.vector.tensor_mul(out=aq[:], in0=yq[:], in1=yq[:])
                    nc.vector.scalar_tensor_tensor(
                        out=aq[:], in0=aq[:], scalar=-0.5, in1=w[:],
                        op0=OP.mult, op1=OP.mult)
                    nc.vector.scalar_tensor_tensor(
                        out=yq[:], in0=aq[:], scalar=1.5, in1=yq[:],
                        op0=OP.add, op1=OP.mult)
                # nb = -mu * rstd
                nb = wpool.tile([128, 4, 1], f32, tag="nb")
                nc.vector.scalar_tensor_tensor(
                    out=nb[:], in0=mv[:, :, 0:1], scalar=-1.0, in1=yq[:],
                    op0=OP.mult, op1=OP.mult)
                t = wpool.tile([128, 4, DOUT], f32, tag="t")
                for ec in range(4):
                    nc.scalar.activation(out=t[:, ec, :], in_=h[:, ec, :],
                                         func=AF.Identity, bias=nb[:, ec, :],
                                         scale=yq[:, ec, :])
                fin = wpool.tile([128, 4, DOUT], f32, tag="fin")
                if apply_lng:
                    for ec in range(4):
                        nc.gpsimd.tensor_mul(out=t[:, ec, :], in0=t[:, ec, :],
                                             in1=lng_t[:])
                nc.gpsimd.tensor_add(out=fin[:], in0=t[:], in1=dn[:])
                nc.sync.dma_start(out=out_d[gi].rearrange("c p m -> p c m"),
                                  in_=fin[:])

            # PE warmup: keep the HAM activity monitor busy while the first
            # graph's DMAs land so real matmuls start at full clock.
            wup = cpool.tile([128, N], fmm)
            nc.gpsimd.memset(wup[:].bitcast(f32), 0.25)
            for _ in range(44):
                pw = ps1.tile([128, N], f32, tag="ps1")
                mm(pw[:], wup[:, :128], wup[:], start=True, stop=True)

            # software pipeline: loads 2 ahead; front(g+1) before back(g)
            sts = {0: loads(0)}
            if g > 1:
                sts[1] = loads(1)
            prev = front(0, sts[0])
            prev_gi = 0
            for gi in range(1, g):
                if gi + 1 < g:
                    sts[gi + 1] = loads(gi + 1)
                cur = front(gi, sts[gi])
                back(gi - 1, {**sts[prev_gi], **prev})
                del sts[prev_gi]
                prev, prev_gi = cur, gi
            back(g - 1, {**sts[prev_gi], **prev})

    nc.compile()
    _BUILT[key] = nc
    return nc


def tf32_round(a):
    """Round-to-nearest-even fp32 -> tf32 (10-bit mantissa) == fp32r."""
    u = np.ascontiguousarray(a, np.float32).view(np.uint32)
    u = (u + np.uint32(0x0FFF) + ((u >> np.uint32(13)) & np.uint32(1))) \
        & np.uint32(0xFFFFE000)
    return u.view(np.float32)


def prep_host(inputs, adj, op_emb, dgf_W, dgf_b, dgf_opW, dgf_opb,
              Wk, Wv, Wq, a_w, gat_opW, gat_opb, ln_g, ln_b):
    """Fold params + lay out per-graph tensors for the device kernel."""
    f = np.float32
    x = np.asarray(inputs, f)
    adj = np.asarray(adj, f)
    ope = np.asarray(op_emb, f)
    nb = x.shape[0]

    xt = np.ascontiguousarray(x.transpose(0, 2, 1)).reshape(nb, 2, 128, N)
    adjt = np.ascontiguousarray(adj.transpose(0, 2, 1)).reshape(nb, 4, 128, N)
    et = np.ascontiguousarray(ope.transpose(0, 2, 1))  # [nb, 64, N]
    eta = np.concatenate([et, np.ones((nb, 1, N), f)], axis=1)  # [nb, 65, N]

    wcomb = np.ascontiguousarray(np.concatenate(
        [0.25 * np.asarray(dgf_W, f), np.asarray(Wv, f).T],
        axis=1)).reshape(2, 128, 512)
    mq = np.ascontiguousarray(
        (np.asarray(Wq, f).T * np.asarray(a_w, f)[None, :]) @ np.asarray(Wk, f)
        / np.sqrt(np.float32(DOUT))).reshape(2, 128, DIN)
    gcomb = np.ascontiguousarray(np.concatenate([
        np.concatenate([np.asarray(dgf_opW, f).T,
                        np.asarray(dgf_opb, f)[None, :]], 0),
        np.concatenate([np.asarray(gat_opW, f).T,
                        np.asarray(gat_opb, f)[None, :]], 0)], axis=1))
    ch = np.ascontiguousarray(
        (0.5 * (np.asarray(dgf_b, f) + np.asarray(ln_b, f))).reshape(1, DOUT))
    lng = np.ascontiguousarray((0.5 * np.asarray(ln_g, f)).reshape(1, DOUT))
    apply_lng = not (np.all(np.asarray(ln_g, f) == 1.0))
    hp = dict(xt=xt, adjt=adjt, eta=eta, wcomb=wcomb, mq=mq, gcomb=gcomb,
              chalf=ch, lngh=lng)
    if MM_DT == "float32r":
        # matmul-feeding tensors must carry fp32r(=tf32)-rounded values
        for k in ("xt", "adjt", "eta", "wcomb", "mq", "gcomb"):
            hp[k] = tf32_round(hp[k])
    return hp, apply_lng


MM_DT = "float32r"


def run(hp, apply_lng, mm_dt=None, trace=False, **kw):
    from concourse.bass_utils import run_bass_kernel_spmd

    nc = build_bass(G, mm_dt or MM_DT, apply_lng)
    in_maps = []
    for c in range(NCORES):
        sl = slice(c * G, (c + 1) * G)
        m = {k: (v[sl] if k in ("xt", "adjt", "eta") else v)
             for k, v in hp.items()}
        in_maps.append(m)
    res = run_bass_kernel_spmd(nc, in_maps, core_ids=list(range(NCORES)),
                               trace=trace, **kw)
    out = np.concatenate(
        [r["out"].reshape(G, N, DOUT) for r in res.results], axis=0)
    return np.ascontiguousarray(out), res


def kernel(**inputs) -> np.ndarray:
    hp, apply_lng = prep_host(**inputs)
    out, _ = run(hp, apply_lng)
    return out
